# revision 4
# baseline (speedup 1.0000x reference)
"""Causal self-attention (B=2, S=2048, D=2048, 16 heads, RoPE) on 8 trn2 cores.

Sharding: tensor-parallel over heads x data-parallel over batch.
Core c handles batch b = c // 4 and head-group hg = c % 4 (heads 4*hg..4*hg+3).
qkv_proj is column-sharded by head, out_proj row-sharded by head; the
AllReduce of the out_proj partials is done on the host (4 partials per batch).

Per-core device program (all matmuls bf16 with fp32 PSUM accumulation):
  phase 1: qT/kT = W_{q,k} x^T per head-feature tile, RoPE fused into the
           PSUM->SBUF evacuation (DVE); v computed token-major via
           lhsT = x^T tiles.
  phase 2: per head, flash-style over query blocks of 512: S^T tile
           [j=128, i=512] via one matmul (contraction d=128), causal mask
           added on diagonal-crossing blocks, ACT exp (scale folded in),
           P^T bf16; PV accumulation out^T += v_tile^T P^T and the softmax
           denominator via an all-ones lhsT matmul into a [1, 512] PSUM row.
           No max-subtraction: scores*scale are O(6) here, exp is safe in
           fp32 (reference softmax is shift-invariant).
  phase 3: out_partial[t, :] = sum_fc ctx^T[fc, t].T @ woT[fc] tiles.
"""

import math
import os

import numpy as np
import ml_dtypes

import bass_rust
import concourse.bass as bass
import concourse.mybir as mybir
import concourse.tile as tile
from concourse.bass import ts
from concourse.bass_utils import run_bass_kernel_spmd

BF16 = ml_dtypes.bfloat16
F32 = mybir.dt.float32
BF = mybir.dt.bfloat16

B = 2
S = 2048
D = 2048
HD = 128                    # head dim
NH = 16                     # total heads
NHL = 4                     # heads per core
FQ = NHL * HD               # 512 per-core q/k/v features
KC = D // 128               # 16 contraction chunks
TB = 4                      # token blocks of 512 (qkv phase)
IB = 4                      # query blocks of 512 (attention phase)
JT = S // 128               # 16 key tiles of 128
SCALE = 1.0 / math.sqrt(HD)
NEG = -30000.0              # additive mask; exp(NEG * SCALE) == 0 in fp32

MAX_WAITS = 1               # this walrus build allows 1 sync-wait per inst

_wait_ctr = [0]


def _split_all_multi_waits(nc):
    """This walrus build rejects instructions with >1 semaphore wait
    ("Too many sync wait commands").  Move extra waits onto NoOps inserted
    right before the instruction on the same engine (sequencers execute in
    order, so blocking one instruction earlier is equivalent)."""
    n_split = 0
    for f in nc.m.functions:
        for blk in f.blocks:
            out = []
            for inst in blk.instructions:
                si = inst.sync_info
                if si is not None and len(si.on_wait) > MAX_WAITS:
                    waits = list(si.on_wait)
                    for w in waits[:-MAX_WAITS] if MAX_WAITS else waits:
                        _wait_ctr[0] += 1
                        nop = mybir.InstNoOp(
                            name=f"I-waitsplit-{_wait_ctr[0]}", ins=[], outs=[]
                        )
                        nop.engine = inst.engine
                        nop.sync_info = bass_rust.SyncInfo(on_wait=[w], on_update=[])
                        out.append(nop)
                    inst.sync_info = bass_rust.SyncInfo(
                        on_wait=waits[-MAX_WAITS:], on_update=list(si.on_update)
                    )
                    n_split += 1
                out.append(inst)
            blk.instructions = out
    return n_split


def build_nc(reps: int = 1, split_waits: bool = True):
    nc = bass.Bass()
    xT = nc.declare_dram_parameter("xT", [128, KC, S], BF, isOutput=False)
    wqT = nc.declare_dram_parameter("wqT", [128, KC, FQ], BF, isOutput=False)
    wkT = nc.declare_dram_parameter("wkT", [128, KC, FQ], BF, isOutput=False)
    wvT = nc.declare_dram_parameter("wvT", [128, KC, FQ], BF, isOutput=False)
    woT = nc.declare_dram_parameter("woT", [128, NHL, D], BF, isOutput=False)
    cosT = nc.declare_dram_parameter("cosT", [128, S], F32, isOutput=False)
    sinT = nc.declare_dram_parameter("sinT", [128, S], F32, isOutput=False)
    maskd = nc.declare_dram_parameter("maskd", [128, 128], F32, isOutput=False)
    out = nc.declare_dram_parameter("out", [S, D], F32, isOutput=True)

    mult = mybir.AluOpType.mult
    add = mybir.AluOpType.add
    EXP = mybir.ActivationFunctionType.Exp

    with tile.TileContext(nc) as tc:
        with tc.tile_pool(name="persist", bufs=1) as persist:
            qT = persist.tile([128, NHL, S], BF, tag="qT")
            kT = persist.tile([128, NHL, S], BF, tag="kT")
            vv = persist.tile([128, JT, FQ], BF, tag="vv")   # [t-part, tt, d]
            ctx = persist.tile([128, NHL, S], BF, tag="ctx")
            cos_sb = persist.tile([128, S], F32, tag="cos")
            sin_sb = persist.tile([128, S], F32, tag="sin")
            mask_sb = persist.tile([128, 128], F32, tag="mask")
            ones_sb = persist.tile([128, 128], BF, tag="ones")
            nc.sync.dma_start(cos_sb[:], cosT[:])
            nc.sync.dma_start(sin_sb[:], sinT[:])
            nc.sync.dma_start(mask_sb[:], maskd[:])
            nc.vector.memset(ones_sb[:], 1.0)

            for _rep in range(reps):
                # ---------------- phase 1: qkv + rope ----------------
                with (
                    tc.tile_pool(name="p1x", bufs=2) as p1x,
                    tc.tile_pool(name="p1w", bufs=1) as p1w,
                    tc.tile_pool(name="p1t", bufs=4) as p1t,
                    tc.tile_pool(name="ps1", bufs=4, space="PSUM") as ps1,
                ):
                    wq_sb = p1w.tile([128, KC, FQ], BF, tag="wq")
                    wk_sb = p1w.tile([128, KC, FQ], BF, tag="wk")
                    wv_sb = p1w.tile([128, KC, FQ], BF, tag="wv")
                    nc.sync.dma_start(wq_sb[:], wqT[:])
                    nc.sync.dma_start(wk_sb[:], wkT[:])
                    nc.sync.dma_start(wv_sb[:], wvT[:])

                    for tb in range(TB):
                        tbs = ts(tb, 512)
                        xs = p1x.tile([128, KC, 512], BF, tag="xs")
                        nc.sync.dma_start(xs[:], xT[:, :, tbs])
                        for w_sb, dstT in ((wq_sb, qT), (wk_sb, kT)):
                            for f in range(NHL):
                                ps = ps1.tile([128, 512], F32, tag="ps")
                                for kc in range(KC):
                                    nc.tensor.matmul(
                                        ps[:],
                                        w_sb[:, kc, ts(f, 128)],
                                        xs[:, kc, :],
                                        start=(kc == 0),
                                        stop=(kc == KC - 1),
                                    )
                                # rope: dst = ps*cos + swap(ps)*sin_signed
                                t1 = p1t.tile([128, 512], F32, tag="t1")
                                nc.vector.tensor_tensor(
                                    t1[:], ps[:], cos_sb[:, tbs], mult
                                )
                                t2 = p1t.tile([128, 512], F32, tag="t2")
                                nc.vector.tensor_tensor(
                                    t2[0:64, :], ps[64:128, :], sin_sb[0:64, tbs], mult
                                )
                                nc.vector.tensor_tensor(
                                    t2[64:128, :], ps[0:64, :], sin_sb[64:128, tbs], mult
                                )
                                nc.vector.tensor_tensor(
                                    dstT[:, f, tbs], t1[:], t2[:], add
                                )
                        for s4 in range(4):
                            tt = tb * 4 + s4
                            ps = ps1.tile([128, 512], F32, tag="ps")
                            for kc in range(KC):
                                nc.tensor.matmul(
                                    ps[:],
                                    xs[:, kc, ts(s4, 128)],
                                    wv_sb[:, kc, :],
                                    start=(kc == 0),
                                    stop=(kc == KC - 1),
                                )
                            nc.scalar.copy(vv[:, tt, :], ps[:])

                # ---------------- phase 2: attention ----------------
                with (
                    tc.tile_pool(name="p2p", bufs=4) as p2p,
                    tc.tile_pool(name="p2l", bufs=2) as p2l,
                    tc.tile_pool(name="ps_st", bufs=3, space="PSUM") as ps_st,
                    tc.tile_pool(name="ps_o", bufs=2, space="PSUM") as ps_o,
                    tc.tile_pool(name="ps_l", bufs=2, space="PSUM") as ps_l,
                ):
                    for h in range(NHL):
                        for ib in range(IB):
                            o_ps = ps_o.tile([128, 512], F32, tag="o")
                            l_ps = ps_l.tile([128, 512], F32, tag="l")
                            njt = 4 * ib + 4
                            for jt in range(njt):
                                st = ps_st.tile([128, 512], F32, tag="st")
                                nc.tensor.matmul(
                                    st[:],
                                    kT[:, h, ts(jt, 128)],
                                    qT[:, h, ts(ib, 512)],
                                    start=True,
                                    stop=True,
                                )
                                off = (jt - 4 * ib) * 128
                                pt = p2p.tile([128, 512], BF, tag="pt")
                                if off >= 0:
                                    nc.vector.tensor_tensor(
                                        st[:, off : off + 128],
                                        st[:, off : off + 128],
                                        mask_sb[:],
                                        add,
                                    )
                                    if off > 0:
                                        nc.vector.memset(pt[:, 0:off], 0.0)
                                    nc.scalar.activation(
                                        pt[:, off:512], st[:, off:512], EXP,
                                        scale=SCALE,
                                    )
                                else:
                                    nc.scalar.activation(
                                        pt[:], st[:], EXP, scale=SCALE
                                    )
                                nc.tensor.matmul(
                                    l_ps[:], ones_sb[:], pt[:],
                                    start=(jt == 0), stop=(jt == njt - 1),
                                )
                                nc.tensor.matmul(
                                    o_ps[:], vv[:, jt, ts(h, 128)], pt[:],
                                    start=(jt == 0), stop=(jt == njt - 1),
                                )
                            linv = p2l.tile([128, 512], F32, tag="linv")
                            nc.vector.reciprocal(linv[:], l_ps[:])
                            nc.vector.tensor_tensor(
                                ctx[:, h, ts(ib, 512)], o_ps[:], linv[:], mult
                            )

                # ---------------- phase 3: out proj ----------------
                with (
                    tc.tile_pool(name="p3w", bufs=1) as p3w,
                    tc.tile_pool(name="p3s", bufs=4) as p3s,
                    tc.tile_pool(name="ps3", bufs=4, space="PSUM") as ps3,
                ):
                    wo_sb = p3w.tile([128, NHL, D], BF, tag="wo")
                    nc.sync.dma_start(wo_sb[:], woT[:])
                    for tt in range(JT):
                        for ob in range(4):
                            ps = ps3.tile([128, 512], F32, tag="ps")
                            for fc in range(NHL):
                                nc.tensor.matmul(
                                    ps[:],
                                    ctx[:, fc, ts(tt, 128)],
                                    wo_sb[:, fc, ts(ob, 512)],
                                    start=(fc == 0),
                                    stop=(fc == NHL - 1),
                                )
                            stage = p3s.tile([128, 512], F32, tag="stage")
                            nc.scalar.copy(stage[:], ps[:])
                            nc.sync.dma_start(
                                out[ts(tt, 128), ts(ob, 512)], stage[:]
                            )

    if split_waits:
        _split_all_multi_waits(nc)
    return nc


def _rope_tables():
    inv_freq = 1.0 / (10000.0 ** (np.arange(0, HD, 2, dtype=np.float32) / HD))
    t = np.arange(S, dtype=np.float32)
    freqs = np.einsum("i,j->ij", t, inv_freq)          # [S, 64]
    emb = np.concatenate([freqs, freqs], axis=-1)      # [S, 128]
    cos = np.cos(emb).T.astype(np.float32)             # [128, S]
    sin = np.sin(emb).T.astype(np.float32)             # [128, S]
    sin_signed = sin.copy()
    sin_signed[:64] *= -1.0                            # rotate_half sign fold
    return np.ascontiguousarray(cos), np.ascontiguousarray(sin_signed)


def _mask_diag():
    jj = np.arange(128)[:, None]
    ii = np.arange(128)[None, :]
    return np.where(ii >= jj, 0.0, NEG).astype(np.float32)


def _chunk_pmajor(a):
    """[R, C] with R = n*128 -> [128, n, C] with out[p, n, c] = a[n*128+p, c]."""
    n = a.shape[0] // 128
    return np.ascontiguousarray(a.reshape(n, 128, -1).transpose(1, 0, 2))


def make_in_maps(x, w_qkv, w_out):
    cos, sin_signed = _rope_tables()
    mask = _mask_diag()
    in_maps = []
    xT_by_b = []
    for b in range(B):
        # xT[p, kc, t] = x[b, t, kc*128+p]
        xT_by_b.append(_chunk_pmajor(x[b].T.astype(np.float32)).astype(BF16))
    for c in range(8):
        b, hg = c // 4, c % 4
        rows = slice(hg * FQ, (hg + 1) * FQ)
        wq = _chunk_pmajor(w_qkv[0 * D:][rows].T).astype(BF16)   # [128, KC, FQ]
        wk = _chunk_pmajor(w_qkv[1 * D:][rows].T).astype(BF16)
        wv = _chunk_pmajor(w_qkv[2 * D:][rows].T).astype(BF16)
        wo = _chunk_pmajor(w_out[:, hg * FQ:(hg + 1) * FQ].T).astype(BF16)
        in_maps.append(
            {
                "xT": xT_by_b[b],
                "wqT": wq,
                "wkT": wk,
                "wvT": wv,
                "woT": wo,
                "cosT": cos,
                "sinT": sin_signed,
                "maskd": mask,
            }
        )
    return in_maps


_nc_cache = {}


def kernel(x, w_qkv, w_out):
    x = np.asarray(x)
    w_qkv = np.asarray(w_qkv)
    w_out = np.asarray(w_out)
    reps = int(os.environ.get("KERNEL_REPS", "1"))
    if reps not in _nc_cache:
        _nc_cache[reps] = build_nc(reps)
    nc = _nc_cache[reps]
    in_maps = make_in_maps(x, w_qkv, w_out)
    res = run_bass_kernel_spmd(nc, in_maps, list(range(8)), trace=False)
    out = np.zeros((B, S, D), dtype=np.float32)
    for c in range(8):
        out[c // 4] += res.results[c]["out"]
    return out
